# revision 25
# baseline (speedup 1.0000x reference)
"""HGT (heterogeneous graph transformer) 2-layer forward on 8 Trainium2 cores.

Strategy (edge/dst sharding):
 - Nodes are permuted and dealt to 8 cores (1250 drug + 1250 disease + 3750
   protein each, padded to 1280/1280/3840 = 6400 slots so every 128-node tile
   is type-pure). Each core owns the edges whose dst lands in its slice.
 - Node front (per core): k/v projections for all 51200 node slots into a
   DRAM fp8-e4m3 [k|v] table (256B rows, gathered by src) via one merged
   fp8x fp8[Wk|Wv] matmul per 128-node tile (weights pre-scaled x8, rescaled
   during PSUM evacuation) and ONE bulk DMA per source-core slice; qa = x @
   (Wq . blockdiag(rel_att^T) . pri/sqrt(D)) for all 4 etypes in one
   [QA0|..|QA3] bf16 matmul per tile, rows at loc*4+g. qa and the first 4
   kv slices gate the lo-half gathers (split fence) so the edge phase starts
   before the full front finishes.
 - Edge phase: edges grouped by (src-half @ 25600, etype, dst-degree-bucket)
   into 128-edge tiles where a dst node's edges occupy d consecutive
   partitions. Per 8-tile gather call (int16 dma_gather, 4 SWDGE queues
   round-robin): a = sum16(ke*qa), ea = exp(a)*mask batched over the call;
   W = [v*ea | ea] per tile; a constant block-sum matmul S_d^T @ W reduces
   each node's edges; f32 partials dma_scatter_add (CCE add) into an hv
   table at row node*4+etype. Row-unique-per-half; cross-half CCE races are
   prevented by per-call row-set-intersection dependencies (no global
   lo/hi barrier).
 - Node epilogue: bulk-load 4 chunks of hv rows per DMA, esum over etypes
   batched, reciprocal, per (chunk, etype): hvn = hv_g * recip, PE
   transpose, o^T accumulated as sum_g M2[g,t]^T @ hvn^T with
   M2 = blockdiag(rel_msg)@Wa (one PSUM accumulation group per region),
   sigmoid-skip blend fused into one scalar_tensor_tensor, single bulk
   x_new^T write.
 - x_new is exchanged between the two layer launches on the host.
"""
import numpy as np
import ml_dtypes

H, D, IN = 8, 16, 128
NT, ET = 3, 4
N_DRUG, N_DIS, N_PROT = 10000, 10000, 30000
N = N_DRUG + N_DIS + N_PROT
E = 400000
NCORES = 8
SQRT_D = 4.0

TCNT = (N_DRUG // NCORES, N_DIS // NCORES, N_PROT // NCORES)
TPAD = tuple(-(-c // 128) * 128 for c in TCNT)          # 1280,1280,3840
NLOC = sum(TPAD)                                        # 6400
NSLOT = NCORES * NLOC                                   # 51200
LO_LIMIT = 4 * 6400                                     # lo/hi gather split (int16-safe)
HV_F32 = True      # f32 hv table: CCE f32 scatter-add is ~2x faster than bf16
KV_FP8 = True      # fp8 e4m3 kv table + fp8 x/W for the kv matmuls
HVROW = 192 if HV_F32 else 256                          # elems per hv row
HVE = 136                                               # used elems
HV_ROWS = 4 * NLOC + 128                                # + junk strip
JUNK = 4 * NLOC
CN = 8                                                  # tiles per gather call
SC_B = 8                                                # batches per scatter
EPS_ESUM = 1e-30
DCLS = (1, 2, 4, 8, 16, 32, 64, 128)
NJT = NLOC // 128                                       # 50 node tiles/core
QW = 10                                                 # qa tiles per write
EG = 4                                                  # chunks per epi group
TSEG = ((0, TPAD[0], 0), (TPAD[0], TPAD[0] + TPAD[1], 1),
        (TPAD[0] + TPAD[1], NLOC, 2))                   # node-type segments

BF16 = ml_dtypes.bfloat16
F8E4 = ml_dtypes.float8_e4m3
KV_SCALE = 8.0


def _bf(x):
    return np.ascontiguousarray(np.asarray(x).astype(BF16))


# ---------------------------------------------------------------- host prep
def _host_prep(inp):
    src = np.asarray(inp["src"]).astype(np.int64)
    dst = np.asarray(inp["dst"]).astype(np.int64)
    etype = np.asarray(inp["etype"]).astype(np.int64)
    x0 = np.concatenate([np.asarray(inp["drug_feature"]),
                         np.asarray(inp["disease_feature"]),
                         np.asarray(inp["protein_feature"])], 0).astype(np.float32)

    Wk = np.asarray(inp["Wk"], np.float32)
    Wq = np.asarray(inp["Wq"], np.float32)
    Wv = np.asarray(inp["Wv"], np.float32)
    Wa = np.asarray(inp["Wa"], np.float32)
    rel_att = np.asarray(inp["rel_att"], np.float32)
    rel_msg = np.asarray(inp["rel_msg"], np.float32)
    pri = np.asarray(inp["pri"], np.float32)
    skip = np.asarray(inp["skip"], np.float32)

    BDQ = np.zeros((ET, IN, IN), np.float32)
    BDM = np.zeros((ET, IN, IN), np.float32)
    for g in range(ET):
        for h in range(H):
            sl = slice(h * D, (h + 1) * D)
            BDQ[g][sl, sl] = rel_att[h, g].T * (pri[h, g] / SQRT_D)
            BDM[g][sl, sl] = rel_msg[h, g]
    QAW = np.einsum("tio,goj->tgij", Wq, BDQ)
    M2 = np.einsum("gij,tjo->gtio", BDM, Wa)
    alpha = 1.0 / (1.0 + np.exp(-skip))

    # node -> slot assignment
    deg_tot = np.bincount(dst, minlength=N)
    old_of_slot = np.full(NSLOT, -1, np.int64)
    tbase = (0, TPAD[0], TPAD[0] + TPAD[1])
    for t, (lo, cnt) in enumerate(
            zip((0, N_DRUG, N_DRUG + N_DIS), (N_DRUG, N_DIS, N_PROT))):
        ids = np.arange(lo, lo + cnt)
        ids = ids[np.argsort(-deg_tot[ids], kind="stable")]
        percore = [[] for _ in range(NCORES)]
        for i, nid in enumerate(ids):
            r, k = divmod(i, NCORES)
            c = k if r % 2 == 0 else NCORES - 1 - k
            percore[c].append(nid)
        for c in range(NCORES):
            arr = np.sort(np.array(percore[c], np.int64))
            o = c * NLOC + tbase[t]
            old_of_slot[o:o + len(arr)] = arr
    slot_of_old = np.full(N, -1, np.int64)
    real = old_of_slot >= 0
    slot_of_old[old_of_slot[real]] = np.nonzero(real)[0]

    xs = np.zeros((NSLOT, IN), np.float32)
    xs[real] = x0[old_of_slot[real]]

    slot_t = np.zeros(NLOC, np.int64)
    slot_t[TPAD[0]:TPAD[0] + TPAD[1]] = 1
    slot_t[TPAD[0] + TPAD[1]:] = 2

    # edges in slot space
    e_src = slot_of_old[src]
    e_loc_all = slot_of_old[dst]
    e_core = e_loc_all // NLOC
    e_loc = e_loc_all % NLOC
    e_half = (e_src >= LO_LIMIT).astype(np.int64)

    # group edges by (core, half, g, local dst) -> degree buckets
    node_entries = [[[{d: [] for d in DCLS} for _ in range(ET)] for _ in range(2)]
                    for _ in range(NCORES)]
    order = np.lexsort((e_loc, etype, e_half, e_core))
    oc, oh, og, ol = (e_core[order], e_half[order], etype[order], e_loc[order])
    i, M = 0, len(order)
    while i < M:
        j = i
        c, hf, g, n = oc[i], oh[i], og[i], ol[i]
        while j < M and oc[j] == c and oh[j] == hf and og[j] == g and ol[j] == n:
            j += 1
        k = j - i
        assert k <= 128, f"degree {k} > 128 unsupported"
        d = next(dd for dd in DCLS if dd >= k)
        node_entries[c][hf][g][d].append((n, order[i:j]))
        i = j

    ntile = {}
    for hf in range(2):
        for g in range(ET):
            for d in DCLS:
                cap = 128 // d
                mx = max(len(node_entries[c][hf][g][d]) for c in range(NCORES))
                ntile[(hf, g, d)] = -(-mx // cap) if mx else 0

    tiles = []
    for hf in range(2):
        for g in range(ET):
            for d in DCLS:
                tiles += [(hf, g, d)] * ntile[(hf, g, d)]
    T = len(tiles)
    n_lo_tiles = sum(1 for t in tiles if t[0] == 0)

    kv_idx = np.zeros((NCORES, T, 128), np.int32)
    qa_idx = np.zeros((NCORES, T, 128), np.int32)
    mask = np.zeros((NCORES, T, 128), np.float32)
    sc_row = np.full((NCORES, T, 128), JUNK, np.int32)

    for c in range(NCORES):
        ti = 0
        for hf in range(2):
            for g in range(ET):
                for d in DCLS:
                    nt = ntile[(hf, g, d)]
                    if nt == 0:
                        continue
                    cap = 128 // d
                    ents = node_entries[c][hf][g][d]
                    for tt in range(nt):
                        for s, (n, eids) in enumerate(
                                ents[tt * cap:(tt + 1) * cap]):
                            sc_row[c, ti + tt, s] = n * 4 + g
                            p0 = s * d
                            for k2, eid in enumerate(eids):
                                kv_idx[c, ti + tt, p0 + k2] = (
                                    e_src[eid] - (LO_LIMIT if hf else 0))
                                qa_idx[c, ti + tt, p0 + k2] = (
                                    4 * e_loc[eid] + g)
                                mask[c, ti + tt, p0 + k2] = 1.0
                    ti += nt
        assert ti == T

    # gather calls (never cross the lo/hi boundary)
    def _chunks(lo, hi):
        return [(t0, min(t0 + CN, hi)) for t0 in range(lo, hi, CN)]
    g_calls = _chunks(0, n_lo_tiles) + _chunks(n_lo_tiles, T)

    # tile -> (call index, slot within call)
    tile_call = {}
    for ci, (t0, t1) in enumerate(g_calls):
        for j in range(t1 - t0):
            tile_call[t0 + j] = (ci, j)

    # PSUM batches
    batches = []
    i = 0
    while i < T:
        hf, g, d = tiles[i]
        if d == 1:
            batches.append({"d1": True, "tiles": [i]})
            i += 1
            continue
        per = 2 if d == 2 else 3
        grp = []
        while len(grp) < per and i < T and tiles[i] == (hf, g, d):
            grp.append(i)
            i += 1
        step = 64 if d == 2 else 32
        batches.append({"d1": False, "d": d, "tiles": grp, "step": step,
                        "qoffs": [k * step for k in range(len(grp))]})
    NB = len(batches)

    def bat_half(bi):
        return tiles[batches[bi]["tiles"][0]][0]

    sc_calls = []
    i = 0
    while i < NB:
        j = i
        while j < NB and j - i < SC_B and bat_half(j) == bat_half(i):
            j += 1
        sc_calls.append((i, j))
        i = j

    sc_idx = np.full((NCORES, NB, 128), JUNK, np.int32)
    for c in range(NCORES):
        for bi, b in enumerate(batches):
            if b["d1"]:
                sc_idx[c, bi, :] = sc_row[c, b["tiles"][0], :]
            else:
                d, step = b["d"], b["step"]
                m = 128 // d
                for (tidx, qoff) in zip(b["tiles"], b["qoffs"]):
                    sc_idx[c, bi, qoff:qoff + m] = sc_row[c, tidx, :m]

    def wrap16(flat):
        a = flat.astype(np.int16).reshape(-1, 16).T
        return np.tile(a, (8, 1))

    def build_gather_idx(arr):
        return np.concatenate(
            [wrap16(arr[t0:t1].reshape(-1)) for (t0, t1) in g_calls], 1)

    kv_idx_w = np.stack([build_gather_idx(kv_idx[c]) for c in range(NCORES)])
    qa_idx_w = np.stack([build_gather_idx(qa_idx[c]) for c in range(NCORES)])
    sc_idx_w = np.stack([
        np.concatenate([wrap16(sc_idx[c, b0:b1].reshape(-1))
                        for (b0, b1) in sc_calls], 1)
        for c in range(NCORES)])
    mask_bf = np.stack([_bf(mask[c].T) for c in range(NCORES)])

    # const block
    blocks, cmap = [], {}

    def add_c(name, mat):
        cmap[name] = (sum(b.shape[1] for b in blocks), mat.shape[1])
        blocks.append(_bf(mat))
    blocks8, cmap8 = [], {}

    def add_c8(name, mat):
        cmap8[name] = (sum(b.shape[1] for b in blocks8), mat.shape[1])
        blocks8.append(np.ascontiguousarray(
            (np.asarray(mat) * KV_SCALE).astype(F8E4)))
    for t in range(NT):
        add_c8(f"KV{t}", np.concatenate([Wk[t], Wv[t]], 1))
    for t in range(NT):
        add_c(f"QA{t}", np.concatenate([QAW[t, g] for g in range(ET)], 1))
    for g in range(ET):
        for t in range(NT):
            add_c(f"M2{g}{t}", M2[g, t])
    for d in DCLS[1:]:
        m = 128 // d
        w = 64 if d == 2 else 32
        S = np.zeros((128, w), np.float32)
        for s in range(m):
            S[s * d:(s + 1) * d, s] = 1.0
        add_c(f"S{d}", S)
    add_c("Szero", np.zeros((128, 64), np.float32))
    add_c("ident", np.eye(128, dtype=np.float32))
    wconst = np.concatenate(blocks, 1)
    wconst8 = np.concatenate(blocks8, 1)

    # fine-grained lo/hi scatter dependencies: hi call -> lo calls whose
    # real (non-junk) row sets intersect (union over cores; SPMD program)
    lo_calls_idx = [i for i, (b0, b1) in enumerate(sc_calls)
                    if bat_half(b0) == 0]
    hi_calls_idx = [i for i, (b0, b1) in enumerate(sc_calls)
                    if bat_half(b0) == 1]
    call_rows = []
    for (b0, b1) in sc_calls:
        rs = set()
        for c in range(NCORES):
            v = sc_idx[c, b0:b1, :].reshape(-1)
            rs.update(int(x) for x in v[v != JUNK])
        call_rows.append(rs)
    sc_deps = {}
    for hi_i in hi_calls_idx:
        deps = [lo_i for lo_i in lo_calls_idx
                if call_rows[hi_i] & call_rows[lo_i]]
        sc_deps[hi_i] = deps
    meta = {"tiles": tiles, "g_calls": g_calls, "batches": batches,
            "sc_deps": sc_deps,
            "sc_calls": sc_calls, "n_lo_tiles": n_lo_tiles, "cmap": cmap,
            "wcols": wconst.shape[1], "wcols8": wconst8.shape[1],
            "cmap8": cmap8, "slot_t": slot_t, "alpha": alpha,
            "T": T, "NB": NB, "tile_call": tile_call}
    percore = {"kv_idx": kv_idx_w, "qa_idx": qa_idx_w, "sc_idx": sc_idx_w,
               "mask": mask_bf,
               "xTloc": np.stack([_bf(xs[c * NLOC:(c + 1) * NLOC].T)
                                  for c in range(NCORES)])}
    shared = {"xT": np.concatenate(
                  [_bf(xs[c * NLOC:(c + 1) * NLOC].T) for c in range(NCORES)], 0),
              "wconst": wconst, "wconst8": wconst8}
    asm = {"old_of_slot": old_of_slot, "real": real}
    return meta, percore, shared, asm


# ---------------------------------------------------------------- bass build
def _build(meta, last_layer, repeats=1, debug_dump=False, phases=('front', 'edge', 'epi')):
    import contextlib
    import concourse.bacc as bacc
    import concourse.mybir as mybir
    import concourse.tile as tile
    from concourse import library_config
    from concourse.tile_rust import add_dep_helper

    f32 = mybir.dt.float32
    bf16 = mybir.dt.bfloat16
    i16 = mybir.dt.int16
    AX = mybir.AxisListType.X
    AF = mybir.ActivationFunctionType
    ALU = mybir.AluOpType

    tiles = meta["tiles"]
    g_calls = meta["g_calls"]
    batches = meta["batches"]
    sc_calls = meta["sc_calls"]
    cmap = meta["cmap"]
    cmap8 = meta["cmap8"]
    WCOLS = meta["wcols"]
    WCOLS8 = meta["wcols8"]
    slot_t = meta["slot_t"]
    alpha = meta["alpha"]
    T = meta["T"]
    NB = meta["NB"]
    tile_call = meta["tile_call"]
    sc_deps = meta["sc_deps"]

    IDXW = sum(8 * (t1 - t0) for (t0, t1) in g_calls)
    SCW = NB * 8

    nc = bacc.Bacc("TRN2", target_bir_lowering=False, debug=False,
                   num_swdge_queues=4)

    f8 = mybir.dt.float8e4
    xT_in = nc.dram_tensor("xT8", [NCORES * IN, NLOC], f8, kind="ExternalInput")
    xTloc_in = nc.dram_tensor("xTloc", [IN, NLOC], bf16, kind="ExternalInput")
    wconst_in = nc.dram_tensor("wconst", [128, WCOLS], bf16, kind="ExternalInput")
    wconst8_in = nc.dram_tensor("wconst8", [128, WCOLS8], f8, kind="ExternalInput")
    kvidx_in = nc.dram_tensor("kvidx", [128, IDXW], i16, kind="ExternalInput")
    qaidx_in = nc.dram_tensor("qaidx", [128, IDXW], i16, kind="ExternalInput")
    scidx_in = nc.dram_tensor("scidx", [128, SCW], i16, kind="ExternalInput")
    mask_in = nc.dram_tensor("mask", [128, T], bf16, kind="ExternalInput")
    out_dt = f32 if last_layer else bf16
    out_t = nc.dram_tensor("out", [IN, NLOC], out_dt, kind="ExternalOutput")

    dbg_kind = "ExternalOutput" if debug_dump else "Internal"
    kvdt = mybir.dt.float8e4 if KV_FP8 else bf16
    kv_tbl = nc.dram_tensor("kv_tbl", [NSLOT, 2 * IN], kvdt, kind=dbg_kind)
    qa_tbl = nc.dram_tensor("qa_tbl", [NLOC, ET * IN], bf16, kind=dbg_kind)
    hvdt = f32 if HV_F32 else bf16
    hv1 = nc.dram_tensor("hv1", [HV_ROWS, HVROW], hvdt, kind=dbg_kind)

    import concourse.bass as bass

    with tile.TileContext(nc) as tc, contextlib.ExitStack() as ctx:
        lib_inst = nc.gpsimd.load_library(library_config.mlp)

        consts = ctx.enter_context(tc.tile_pool(name="consts", bufs=1))
        wsb = consts.tile([128, WCOLS], bf16)
        nc.sync.dma_start(out=wsb[:, :], in_=wconst_in[:, :])

        def cst(name):
            off, w = cmap[name]
            return wsb[:, off:off + w]

        wsb8 = consts.tile([128, WCOLS8], f8)
        nc.sync.dma_start(out=wsb8[:, :], in_=wconst8_in[:, :])

        def cst8(name):
            off, w = cmap8[name]
            return wsb8[:, off:off + w]

        idx_kv = consts.tile([128, IDXW], i16)
        idx_qa = consts.tile([128, IDXW], i16)
        idx_sc = consts.tile([128, SCW], i16)
        msk = consts.tile([128, T], bf16)
        nc.sync.dma_start(out=idx_kv[:, :], in_=kvidx_in[:, :])
        nc.sync.dma_start(out=idx_qa[:, :], in_=qaidx_in[:, :])
        nc.sync.dma_start(out=idx_sc[:, :], in_=scidx_in[:, :])
        nc.sync.dma_start(out=msk[:, :], in_=mask_in[:, :])

        zt = consts.tile([128, HVROW], hvdt)
        nc.vector.memset(zt[:, :], 0.0)

        def zero_hv(hv):
            insts = []
            step = 4096
            for r in range(0, HV_ROWS, step):
                n = min(step, HV_ROWS - r)
                z2 = zt[:, :HVE]
                src = bass.AP(tensor=z2.tensor, offset=z2.offset,
                              ap=[list(z2.ap[0]), [0, n // 128],
                                  [1, HVE]])
                insts.append(nc.gpsimd.dma_start(
                    out=hv[r:r + n, :HVE].rearrange(
                        "(p a) f -> p a f", a=n // 128),
                    in_=src))
            return insts

        def fence(producers):
            nop = nc.sync.nop()
            for p in producers:
                add_dep_helper(nop.ins, p.ins, reason="fb")
            return nop

        def gate(consumer, nop):
            if nop is not None:
                add_dep_helper(consumer.ins, nop.ins, reason="ff")

        def expand_inner(a, count):
            return bass.AP(tensor=a.tensor, offset=a.offset,
                           ap=[list(x) for x in a.ap] + [[0, count]])

        def cpy(alt, out, in_):
            if alt % 2:
                return nc.scalar.copy(out=out, in_=in_)
            return nc.vector.tensor_copy(out=out, in_=in_)

        zh1 = zero_hv(hv1)

        # ---------------- node front (qa first; kv writes split for the
        # lo/hi gather fences)
        def node_front(xsrc_ap, xloc_sb, gnop, writes_lo, writes_hi):
            with tc.tile_pool(name="nf", bufs=2) as nf, \
                 tc.tile_pool(name="nfp", bufs=4, space="PSUM") as nfp, \
                 tc.tile_pool(name="nfq", bufs=2, space="PSUM") as nfq:
                # qa (local, all 4 etypes per matmul; rows at loc*4+g)
                for qg in range(NJT // QW):
                    qasb = nf.tile([128, QW, ET * IN], bf16, tag="qasb")
                    for u in range(QW):
                        jt = qg * QW + u
                        t = int(slot_t[jt * 128])
                        qps = nfq.tile([128, ET * IN], f32, tag="qps")
                        nc.tensor.matmul(
                            out=qps[:, :],
                            lhsT=xloc_sb[:, jt * 128:(jt + 1) * 128],
                            rhs=cst(f"QA{t}"), start=True, stop=True)
                        cpy(u, qasb[:, u, :], qps[:, :])
                    r0 = qg * QW * 128
                    writes_lo.append(nc.sync.dma_start(
                        out=qa_tbl[r0:r0 + QW * 128, :].rearrange(
                            "(j p) f -> p j f", p=128),
                        in_=qasb[:, :, :]))
                for c2 in range(NCORES):
                    xin = nf.tile([128, NLOC], f8, tag="xin")
                    for hx in range(2):
                        w0 = hx * (NLOC // 2)
                        ld = nc.sync.dma_start(
                            out=xin[:, w0:w0 + NLOC // 2],
                            in_=xsrc_ap[c2 * IN:(c2 + 1) * IN,
                                        w0:w0 + NLOC // 2])
                        gate(ld, gnop)
                    kvsb = nf.tile([128, NJT, 2 * IN], kvdt, tag="kvsb")
                    for jp in range(NJT // 2):
                        kps = nfp.tile([128, 2, 2 * IN], f32, tag="kps")
                        for u in range(2):
                            jt = jp * 2 + u
                            t = int(slot_t[jt * 128])
                            nc.tensor.matmul(
                                out=kps[:, u, :],
                                lhsT=xin[:, jt * 128:(jt + 1) * 128],
                                rhs=cst8(f"KV{t}"), start=True, stop=True)
                        if jp % 2:
                            nc.scalar.mul(out=kvsb[:, jp * 2:jp * 2 + 2, :],
                                          in_=kps[:, :, :], mul=1.0 / KV_SCALE)
                        else:
                            nc.vector.tensor_scalar_mul(
                                out=kvsb[:, jp * 2:jp * 2 + 2, :],
                                in0=kps[:, :, :], scalar1=1.0 / KV_SCALE)
                    base = c2 * NLOC
                    wr = nc.sync.dma_start(
                        out=kv_tbl[base:base + NLOC, :].rearrange(
                            "(j p) f -> p j f", p=128),
                        in_=kvsb[:, :, :])
                    ((writes_lo if base < LO_LIMIT else writes_hi)
                     .append(wr))

        # ---------------- edge phase
        def edge_phase(hv, lo_nop, hi_nop, zh_insts):
            sc_lo, sc_hi = [], []
            qa_src = bass.AP(tensor=qa_tbl, offset=0,
                             ap=[[IN, ET * NLOC], [1, IN]])
            with tc.tile_pool(name="eg", bufs=3) as eg, \
                 tc.tile_pool(name="ew", bufs=4) as ew, \
                 tc.tile_pool(name="est", bufs=3) as est, \
                 tc.tile_pool(name="stgp", bufs=3) as stgp, \
                 tc.tile_pool(name="epsum", bufs=2, space="PSUM") as epsum:

                wcalls = {}
                bat_cursor = [0]
                stg_state = {"tile": None, "k": 0}
                sc_ci = [0]
                sc_by_idx = {}

                def wview(ti):
                    ci, j = tile_call[ti]
                    return wcalls[ci][:, j, :]

                def flush_scatter():
                    st = stg_state
                    if st["tile"] is None or st["k"] == 0:
                        return
                    b0, b1 = sc_calls[sc_ci[0]]
                    assert b1 - b0 == st["k"], (b0, b1, st["k"])
                    nb = st["k"]
                    si = nc.gpsimd.dma_scatter_add(
                        hv[:, :HVE], st["tile"][:, :nb, :],
                        idx_sc[:, b0 * 8:b0 * 8 + nb * 8],
                        128 * nb, 128 * nb, HVE, elem_step=HVROW,
                        queue_num=sc_ci[0] % 4)
                    add_dep_helper(si.ins, lib_inst.ins, reason="lib")
                    for z in zh_insts:
                        add_dep_helper(si.ins, z.ins, reason="zh")
                    if tiles[batches[b0]["tiles"][0]][0] == 0:
                        sc_lo.append(si)
                        sc_by_idx[sc_ci[0]] = si
                    else:
                        sc_hi.append(si)
                        for lo_i in sc_deps.get(sc_ci[0], ()):
                            dep = sc_by_idx.get(lo_i)
                            if dep is not None:
                                add_dep_helper(si.ins, dep.ins, reason="lohi")
                    sc_ci[0] += 1
                    st["tile"] = None
                    st["k"] = 0

                def process_ready_batches(tiles_done):
                    while bat_cursor[0] < NB:
                        b = batches[bat_cursor[0]]
                        if b["tiles"][-1] >= tiles_done:
                            return
                        st = stg_state
                        if st["tile"] is None:
                            st["tile"] = stgp.tile([128, SC_B, HVE], hvdt,
                                                   name="stg", tag="stg")
                        k = st["k"]
                        b0, b1 = sc_calls[sc_ci[0]]
                        if b["d1"]:
                            # batch a run of consecutive d1 tiles from the
                            # same gather call into one copy
                            ti0 = b["tiles"][0]
                            ci0, j0 = tile_call[ti0]
                            max_run = (b1 - b0) - k
                            run = 1
                            while run < max_run and bat_cursor[0] + run < NB:
                                nb_ = batches[bat_cursor[0] + run]
                                if not nb_["d1"]:
                                    break
                                tin = nb_["tiles"][0]
                                if tin >= tiles_done:
                                    break
                                cin, jin = tile_call[tin]
                                if cin != ci0 or jin != j0 + run:
                                    break
                                run += 1
                            cpy(k, st["tile"][:, k:k + run, :],
                                wcalls[ci0][:, j0:j0 + run, :])
                            st["k"] += run
                            bat_cursor[0] += run
                        else:
                            d, step = b["d"], b["step"]
                            ps = epsum.tile([128, HVE], f32, tag="ps")
                            for (tidx, qoff) in zip(b["tiles"], b["qoffs"]):
                                nc.tensor.matmul(
                                    out=ps[qoff:qoff + step, :],
                                    lhsT=cst(f"S{d}")[:, :step],
                                    rhs=wview(tidx),
                                    start=True, stop=True)
                            nstrip = 2 if b["d"] == 2 else 3
                            for k2 in range(len(b["tiles"]), nstrip):
                                nc.tensor.matmul(
                                    out=ps[k2 * step:(k2 + 1) * step, :],
                                    lhsT=cst("Szero")[:, :step],
                                    rhs=wview(b["tiles"][-1]),
                                    start=True, stop=True)
                            if b["d"] >= 4:
                                nc.vector.memset(ps[96:128, :], 0.0)
                            cpy(k, st["tile"][:, k, :], ps[:, :])
                            st["k"] += 1
                            bat_cursor[0] += 1
                        b0, b1 = sc_calls[sc_ci[0]]
                        if st["k"] == b1 - b0:
                            flush_scatter()

                idx_off = 0
                for ci, (t0, t1) in enumerate(g_calls):
                    ntl = t1 - t0
                    lo = tiles[t0][0] == 0
                    kvb = eg.tile([128, CN, 2 * IN], kvdt, tag="kvb")
                    qab = eg.tile([128, CN, IN], bf16, tag="qab")
                    src_ap = (kv_tbl[0:LO_LIMIT, :] if lo
                              else kv_tbl[LO_LIMIT:NSLOT, :])
                    gi = nc.gpsimd.dma_gather(
                        kvb[:, :ntl, :], src_ap,
                        idx_kv[:, idx_off:idx_off + ntl * 8],
                        128 * ntl, 128 * ntl, 2 * IN,
                        queue_num=(2 * ci) % 4)
                    gate(gi, lo_nop if lo else hi_nop)
                    add_dep_helper(gi.ins, lib_inst.ins, reason="lib")
                    gq = nc.gpsimd.dma_gather(
                        qab[:, :ntl, :], qa_src,
                        idx_qa[:, idx_off:idx_off + ntl * 8],
                        128 * ntl, 128 * ntl, IN,
                        queue_num=(2 * ci + 1) % 4)
                    gate(gq, lo_nop)
                    add_dep_helper(gq.ins, lib_inst.ins, reason="lib")
                    idx_off += ntl * 8

                    # batched logits over the whole call
                    tmp = est.tile([128, CN * IN], bf16, tag="tmp")
                    nc.vector.tensor_mul(out=tmp[:, :ntl * IN],
                                         in0=kvb[:, :ntl, 0:IN],
                                         in1=qab[:, :ntl, :])
                    ast = est.tile([128, CN, H], f32, tag="ast")
                    nc.vector.reduce_sum(
                        out=ast[:, :ntl, :],
                        in_=tmp[:, :ntl * IN].rearrange(
                            "p (j h d) -> p (j h) d", h=H, d=D),
                        axis=AX)
                    eat = est.tile([128, CN, H], bf16, tag="eat")
                    nc.scalar.activation(out=eat[:, :ntl, :],
                                         in_=ast[:, :ntl, :], func=AF.Exp,
                                         scale=1.0)
                    wcall = ew.tile([128, CN, HVE], bf16, tag="wt")
                    wcalls[ci] = wcall
                    nc.vector.tensor_mul(
                        out=wcall[:, :ntl, IN:HVE],
                        in0=eat[:, :ntl, :],
                        in1=expand_inner(msk[:, t0:t1], H))
                    for j in range(ntl):
                        nc.vector.tensor_mul(
                            out=wcall[:, j, 0:IN].rearrange(
                                "p (h d) -> p h d", h=H),
                            in0=kvb[:, j, IN:2 * IN].rearrange(
                                "p (h d) -> p h d", h=H),
                            in1=expand_inner(wcall[:, j, IN:HVE], D))
                    process_ready_batches(t1)
                process_ready_batches(T + 1)
                flush_scatter()
                assert bat_cursor[0] == NB
            return sc_lo + sc_hi

        # ---------------- epilogue (node-major, batched hv loads)
        def epilogue(hv, xloc_sb, out_tensor, out_dtype, sc_nop):
            outs = []
            loads = []
            NCH = NLOC // 128
            groups = [(c0, min(EG, NCH - c0)) for c0 in range(0, NCH, EG)]
            with tc.tile_pool(name="epi", bufs=2) as epi, \
                 tc.tile_pool(name="eps", bufs=3) as epsml, \
                 tc.tile_pool(name="epp", bufs=4, space="PSUM") as epp, \
                 tc.tile_pool(name="epo", bufs=1) as epo:
                otile = epo.tile([128, NLOC], out_dtype)
                xl1m = epo.tile([128, NLOC], bf16)
                for (s0, s1, t) in TSEG:
                    nc.scalar.activation(out=xl1m[:, s0:s1],
                                         in_=xloc_sb[:, s0:s1], func=AF.Copy,
                                         scale=float(1.0 - alpha[t]))
                for (c0, nch) in groups:
                    # full hv rows for nch chunks in one DMA:
                    # hvt[p, a, (g f)] = hv[(c0+a)*512 + p*4 + g, f]
                    hvt = epi.tile([128, EG, 4 * HVROW], hvdt, tag="hvt")
                    src = bass.AP(
                        tensor=hv, offset=c0 * 512 * HVROW,
                        ap=[[4 * HVROW, 128], [512 * HVROW, nch],
                            [1, 4 * HVROW]])
                    ld = nc.sync.dma_start(out=hvt[:, :nch, :], in_=src)
                    gate(ld, sc_nop)
                    loads.append(ld)
                    hvv = hvt[:, :, :].rearrange("p a (g f) -> p a g f", g=4)
                    # esum over etypes (batched over the group's chunks)
                    es = epsml.tile([128, EG, H], f32, tag="es")
                    e2 = epsml.tile([128, EG, H], f32, tag="e2")
                    nc.vector.tensor_add(out=es[:, :nch, :],
                                         in0=hvv[:, :nch, 0, IN:HVE],
                                         in1=hvv[:, :nch, 1, IN:HVE])
                    nc.vector.tensor_add(out=e2[:, :nch, :],
                                         in0=hvv[:, :nch, 2, IN:HVE],
                                         in1=hvv[:, :nch, 3, IN:HVE])
                    nc.vector.tensor_add(out=es[:, :nch, :],
                                         in0=es[:, :nch, :],
                                         in1=e2[:, :nch, :])
                    nc.vector.tensor_scalar_add(out=es[:, :nch, :],
                                                in0=es[:, :nch, :],
                                                scalar1=EPS_ESUM)
                    nc.vector.reciprocal(out=es[:, :nch, :],
                                         in_=es[:, :nch, :])
                    for a in range(nch):
                        ch = c0 + a
                        t = int(slot_t[ch * 128])
                        ops = epp.tile([128, 128], f32, tag="ops")
                        for g in range(ET):
                            hvn = epsml.tile([128, 128], bf16, tag="hvn")
                            nc.vector.tensor_mul(
                                out=hvn[:, :].rearrange(
                                    "p (h d) -> p h d", h=H),
                                in0=hvv[:, a, g, 0:IN].rearrange(
                                    "p (h d) -> p h d", h=H),
                                in1=expand_inner(es[:, a, :], D))
                            tp = epp.tile([128, 128], bf16, tag="tp")
                            nc.tensor.transpose(out=tp[:, :], in_=hvn[:, :],
                                                identity=cst("ident"))
                            hvnT = epsml.tile([128, 128], bf16, tag="hvnT")
                            cpy(g, hvnT[:, :], tp[:, :])
                            nc.tensor.matmul(out=ops[:, :],
                                             lhsT=cst(f"M2{g}{t}"),
                                             rhs=hvnT[:, :], start=(g == 0),
                                             stop=(g == 3))
                        nc.vector.scalar_tensor_tensor(
                            out=otile[:, ch * 128:(ch + 1) * 128],
                            in0=ops[:, :], scalar=float(alpha[t]),
                            in1=xl1m[:, ch * 128:(ch + 1) * 128],
                            op0=ALU.mult, op1=ALU.add)
                outs.append(nc.sync.dma_start(out=out_tensor[:, :],
                                              in_=otile[:, :]))
            return outs, loads

        # ================= single layer (optionally repeated for timing)
        prev = None
        prev_hv_done = None
        for rep in range(repeats):
            if rep > 0:
                zh1 = zero_hv(hv1)
                for z in zh1:
                    gate(z, prev_hv_done)
            with tc.tile_pool(name="xp", bufs=1) as xp:
                xloc_sb = xp.tile([128, NLOC], bf16)
                ldx = nc.sync.dma_start(out=xloc_sb[:, :], in_=xTloc_in[:, :])
                gate(ldx, prev)
                writes_lo = [ldx]
                writes_hi = []
                if 'front' in phases:
                    node_front(xT_in[:, :], xloc_sb, prev,
                               writes_lo, writes_hi)
                f_lo = fence(writes_lo)
                f_hi = fence(writes_lo + writes_hi)
                if 'edge' in phases:
                    sc1 = edge_phase(hv1, f_lo, f_hi, zh1)
                else:
                    sc1 = [f_hi]
                f3 = fence(sc1)
                if 'epi' in phases:
                    eps, hv_loads = epilogue(hv1, xloc_sb, out_t, out_dt, f3)
                else:
                    eps, hv_loads = [f3], [f3]
                prev = fence(eps)
                prev_hv_done = fence(hv_loads)

    nc.compile()
    return nc


# ---------------------------------------------------------------- runner
def _in_maps(meta, percore, shared, xT, xTloc_percore):
    xT8 = np.ascontiguousarray(np.asarray(xT, np.float32).astype(F8E4))
    maps = []
    for c in range(NCORES):
        maps.append({
            "xT8": xT8, "xTloc": xTloc_percore[c],
            "wconst": shared["wconst"], "wconst8": shared["wconst8"],
            "kvidx": percore["kv_idx"][c],
            "qaidx": percore["qa_idx"][c], "scidx": percore["sc_idx"][c],
            "mask": percore["mask"][c]})
    return maps


def kernel(**inputs) -> np.ndarray:
    from concourse import bass2jax

    meta, percore, shared, asm = _host_prep(inputs)
    nc1 = _build(meta, last_layer=False)
    nc2 = _build(meta, last_layer=True)

    maps1 = _in_maps(meta, percore, shared, shared["xT"], percore["xTloc"])
    res1 = bass2jax.run_bass_via_pjrt(nc1, maps1, n_cores=NCORES)
    xnew = [np.asarray(res1[c]["out"]) for c in range(NCORES)]   # (128,6400) bf16
    xag = np.concatenate(xnew, 0)                                # (1024,6400)

    maps2 = _in_maps(meta, percore, shared, xag, xnew)
    res2 = bass2jax.run_bass_via_pjrt(nc2, maps2, n_cores=NCORES)

    out = np.zeros((N, IN), np.float32)
    for c in range(NCORES):
        oc = np.asarray(res2[c]["out"]).T
        sl = slice(c * NLOC, (c + 1) * NLOC)
        rl = asm["real"][sl]
        out[asm["old_of_slot"][sl][rl]] = oc[rl]
    return out
